# revision 1
# baseline (speedup 1.0000x reference)
"""MoE FFN (E=8 experts, top-2) — expert-parallel Bass/Tile kernel for 8 TRN2 cores.

Strategy:
  - Host computes the (tiny) router: logits = x @ gate_w.T, top-2 per token,
    renormalized weights.  Token n is dispatched to cores e1(n), e2(n)
    (expert-parallel: core e holds expert e's weights).
  - Device capacity C=1024 tokens per expert (= the perfectly balanced
    N*K/E share).  Overflow tokens beyond 1024 per expert (~1.3% of
    pairs, pure load imbalance) are computed exactly on the host during
    combine, so every device matmul has a full 512-token free dim.
  - All matmul operands fp16: same 1 row/cycle PE rate as f32r on TRN2
    (only fp8 is faster, far outside the 2e-2 error budget), half the
    DMA traffic and SBUF footprint.  Realized rel err ~4.4e-4.
  - mm1 (pass A then pass B over the two 512-token chunks): hT[hc,c] =
    gelu(w1.T @ xgT + b1); 64 chains of 8 matmuls accumulating over dc.
    All w1 tiles stay resident (64KB/partition) so pass B re-reads SBUF.
  - mm2 computes Y^T: for each (dc, c), a chain of 32 matmuls over hc
    with lhsT = natural w2 tiles; the last chain is split in half so its
    eviction pipeline shortens the kernel tail.  Gate scaling and the
    top-2 combine (plus b2) happen on the host in fp32.
  - Engine assignment keeps queues independent: PE = matmuls only,
    scalar(Act) = gelu + w2/b1 DMA ring, sync(SP) = xg/w1/output DMA
    ring, vector(DVE) = PSUM evictions.
  - Head is tuned around the measured DMA pipeline (shared ~360GB/s bus,
    ~0.65us/DMA ring pacing, ~0.9us completion-sem latency): pass A needs
    only w1#0 (two 128KB halves) + xg-c0 (two 512KB quarters, one per
    ring); xg-c1/w1/w2 stream during pass A.  PE runs gap-free (~0.5us
    total stalls) from first matmul to last.
"""

import re

import numpy as np

import bass_rust
import concourse.bass as bass
import concourse.mybir as mybir
import concourse.tile as tile
from concourse import bacc, bass_utils

P = 128
D_MODEL = 1024
D_HID = 4096
E = 8
TOP_K = 2
N_CORES = 8

DC = D_MODEL // P          # 8 d-chunks
HC = D_HID // P            # 32 h-chunks
C = 1024                   # device per-expert token capacity (rest -> host)
NCH = C // 512             # 2 token chunks of 512 (one PSUM bank each)

F32 = mybir.dt.float32
F16 = mybir.dt.float16
MM_DT = F16

W2_BUFS = 3                # w2 stream depth (tiles of [128, 4096] fp16)


_tail_patched = False


def _patch_light_tail():
    """Replace Tile's end-of-context machinery (multi-wait drain + two
    all-engine EVSEM barriers + semaphore range-clears, ~10us on HW) with
    single-wait drains on the sync engine covering every logical proc's final
    tick.  The NEFF is executed once per load in this flow, so semaphores
    need not be recycled."""
    global _tail_patched
    if _tail_patched:
        return
    _tail_patched = True

    def _drain_and_barrier(self, tick_clock, wait_clock):
        gc = tick_clock.global_clock
        ticks = eval(re.match(r"VectorClock\((.*)\)", repr(gc)).group(1))
        n = len(ticks)
        for i, v in enumerate(ticks):
            if v > 0:
                vc = bass_rust.VectorClock(
                    [v if j == i else 0 for j in range(n)])
                w = self.nc.sync.drain()
                wait_clock.add_sem_waits(
                    w.ins,
                    bass_rust.ScopedClock({None: vc}),
                    bass_rust.ScopedClock({}),
                )
        popped = self.nc._tile_sem_poison_stack.pop()
        assert popped is self._sem_poison
    tile.TileContext._drain_and_barrier = _drain_and_barrier


def build_nc():
    _patch_light_tail()
    nc = bacc.Bacc("TRN2", target_bir_lowering=False, debug=False,
                   num_devices=N_CORES)

    # Inputs, pre-tiled on host into consumption order (contiguous DMAs):
    #   xgt [NCH, 2, P, 2048]  xgt[c, q, p, r*512+t] = Xg[c*512+t, (4q+r)*128+p]
    #   w1t [HC, P, DC*P]      w1t[hc, p, dc*128+j] = w1[dc*128+p, hc*128+j]
    #   w2t [DC, P, HC*P]      w2t[dc, p, hc*128+j] = w2[hc*128+p, dc*128+j]
    #   b1t [P, HC]            b1t[p, hc] = b1[hc*128+p]
    # Output:
    #   ygt [D, C]             ygt[d, n] = Y[n, d]   (pre-gate, fp16)
    xgt = nc.dram_tensor("xgt", [NCH, 2, P, 4 * 512], MM_DT, kind="ExternalInput")
    w1t = nc.dram_tensor("w1t", [HC, P, DC * P], MM_DT, kind="ExternalInput")
    w2t = nc.dram_tensor("w2t", [DC, P, HC * P], MM_DT, kind="ExternalInput")
    b1t = nc.dram_tensor("b1t", [P, HC], F32, kind="ExternalInput")
    ygt = nc.dram_tensor("ygt", [D_MODEL, C], MM_DT, kind="ExternalOutput")

    with tile.TileContext(nc) as tc:
        with (
            tc.tile_pool(name="const", bufs=1) as const,
            tc.tile_pool(name="xg", bufs=1) as xg_pool,
            tc.tile_pool(name="w1", bufs=1) as w1_pool,
            tc.tile_pool(name="w2", bufs=W2_BUFS) as w2_pool,
            tc.tile_pool(name="ht", bufs=1) as ht_pool,
            tc.tile_pool(name="yo", bufs=4) as yo_pool,
            tc.tile_pool(name="ps1", bufs=4, space="PSUM") as ps1,
            tc.tile_pool(name="ps2", bufs=4, space="PSUM") as ps2,
        ):
            b1_sb = const.tile([P, HC], F32, name="b1sb")

            # Head DMAs: only what pass A (token chunk c0) needs — w1#0
            # in two 128KB halves plus the 8 xg-c0 tiles — split across
            # the two HWDGE rings in consumption order.  The shared DMA
            # bus is ~360GB/s with ~650ns/DMA ring pacing and ~900ns
            # completion-sem latency, so keeping the head burst small is
            # what lets the first chains stream without starving.
            xg_sb = {}
            w1_sb = {}

            def w1_load(hc, eng):
                t = w1_pool.tile([P, DC * P], MM_DT, name=f"w1_{hc}")
                eng.dma_start(out=t[:], in_=w1t[hc, :, :])
                w1_sb[hc] = t

            def xg_load(c, q, eng):
                # one 512KB DMA per (chunk, dc-quartet): the ring paces
                # DMAs at ~0.65-1us each regardless of size, so fewer,
                # bigger transfers reach the PE sooner
                t = xg_pool.tile([P, 4 * 512], MM_DT, name=f"xg{c}_{q}")
                eng.dma_start(out=t[:], in_=xgt[c, q, :, :])
                xg_sb[(c, q)] = t

            # First pieces are cut small (subtile deps) so the very first
            # matmul waits on just 32KB of w1 + 128KB of xg and the PE
            # starts ~3us earlier, ramping its clock while the rest of
            # the chain's inputs stream in consumption order.
            w1_sb[0] = w1_pool.tile([P, DC * P], MM_DT, name="w1_0")
            t = xg_pool.tile([P, 4 * 512], MM_DT, name="xg0_0")
            nc.scalar.dma_start(out=w1_sb[0][:, :P], in_=w1t[0, :, :P])
            nc.sync.dma_start(out=t[:, :512], in_=xgt[0, 0, :, :512])
            nc.scalar.dma_start(out=w1_sb[0][:, P:], in_=w1t[0, :, P:])
            nc.sync.dma_start(out=t[:, 512:], in_=xgt[0, 0, :, 512:])
            xg_sb[(0, 0)] = t
            nc.scalar.dma_start(out=b1_sb[:], in_=b1t[:, :])
            xg_load(0, 1, nc.sync)
            w1_load(2, nc.scalar)
            w1_load(1, nc.sync)

            # ---- mm1: hT[hc, c] = gelu(w1.T @ xgT + b1) ----
            # Two passes over the 512-token chunks: all 32 c0 chains,
            # then all 32 c1 chains.  All w1 tiles stay resident (64KB/
            # partition) so pass B re-reads them from SBUF; xg-c1 and w2
            # stream in during pass A when the bus is otherwise idle.
            ht_sb = {}
            w2_sb = {}
            for c in range(NCH):
                for hc in range(HC):
                    if c == 0:
                        if hc + 3 < HC:
                            nhc = hc + 3
                            w1_load(nhc, nc.sync if nhc % 2 == 1
                                    else nc.scalar)
                        if hc == 4:
                            xg_load(1, 0, nc.sync)
                        if hc == 6:
                            xg_load(1, 1, nc.scalar)
                        if hc in (14, 19, 24):
                            # w2 prefetch, spread out so the 1MB transfers
                            # never back up the w1 stream on the same ring
                            k = {14: 0, 19: 1, 24: 2}[hc]
                            t = w2_pool.tile([P, HC * P], MM_DT,
                                             name="w2sb")
                            nc.scalar.dma_start(out=t[:], in_=w2t[k, :, :])
                            w2_sb[k] = t
                    ps = ps1.tile([P, 512], F32, name="ps1")
                    for dc in range(DC):
                        nc.tensor.matmul(
                            ps[:],
                            lhsT=w1_sb[hc][:, dc * P:(dc + 1) * P],
                            rhs=xg_sb[(c, dc // 4)][
                                :, (dc % 4) * 512:(dc % 4 + 1) * 512],
                            start=(dc == 0),
                            stop=(dc == DC - 1),
                        )
                    ht = ht_pool.tile([P, 512], MM_DT, name=f"ht{hc}_{c}")
                    nc.scalar.activation(
                        ht[:], ps[:],
                        mybir.ActivationFunctionType.Gelu,
                        bias=b1_sb[:, hc:hc + 1],
                    )
                    ht_sb[(hc, c)] = ht

            # ---- mm2: Y^T[dc, c] = sum_hc w2tile.T @ hT[hc, c] ----
            for dc in range(DC):
                if dc + W2_BUFS < DC:
                    ndc = dc + W2_BUFS
                    t = w2_pool.tile([P, HC * P], MM_DT, name="w2sb")
                    nc.scalar.dma_start(out=t[:], in_=w2t[ndc, :, :])
                    w2_sb[ndc] = t
                for c in range(NCH):
                    last = (dc == DC - 1 and c == NCH - 1)
                    # The very last chain is split into two half-width
                    # chains so the first half's evict+DMA (~2.2us of
                    # fixed DGE+sem latency) overlaps the second half's
                    # matmuls, shortening the kernel tail.
                    splits = ((0, 256), (256, 256)) if last else ((0, 512),)
                    for s0, sw in splits:
                        ps = ps2.tile([P, 512], F32, name="ps2")
                        for hc in range(HC):
                            nc.tensor.matmul(
                                ps[:, :sw],
                                lhsT=w2_sb[dc][:, hc * P:(hc + 1) * P],
                                rhs=ht_sb[(hc, c)][:, s0:s0 + sw],
                                start=(hc == 0),
                                stop=(hc == HC - 1),
                            )
                        yo = yo_pool.tile([P, 512], MM_DT, name="yo")
                        nc.vector.tensor_scalar_mul(
                            yo[:, :sw], ps[:, :sw], 1.0)
                        nc.sync.dma_start(
                            out=ygt[dc * P:(dc + 1) * P,
                                    c * 512 + s0:c * 512 + s0 + sw],
                            in_=yo[:, :sw],
                        )
    nc.compile()
    return nc


_NC_CACHE = None
TRACE = False
LAST_RESULTS = None


def _get_nc():
    global _NC_CACHE
    if _NC_CACHE is None:
        _NC_CACHE = build_nc()
    return _NC_CACHE


def _erf(x):
    try:
        from scipy.special import erf
        return erf(x)
    except ImportError:
        import math
        return np.frompyfunc(math.erf, 1, 1)(x).astype(np.float64)


def kernel(x, gate_w, w1, b1, w2, b2):
    x = np.asarray(x, dtype=np.float32)
    gate_w = np.asarray(gate_w, dtype=np.float32)
    w1 = np.asarray(w1, dtype=np.float32)
    b1 = np.asarray(b1, dtype=np.float32)
    w2 = np.asarray(w2, dtype=np.float32)
    b2 = np.asarray(b2, dtype=np.float32)

    B, T, D = x.shape
    N = B * T
    xf = x.reshape(N, D)

    # ---- router (host; 0.05% of model FLOPs — the sharding decision) ----
    logits = xf @ gate_w.T                           # [N, E]
    order = np.argsort(-logits, axis=1, kind="stable")
    i1, i2 = order[:, 0], order[:, 1]
    l1 = logits[np.arange(N), i1].astype(np.float64)
    l2 = logits[np.arange(N), i2].astype(np.float64)
    g1 = (1.0 / (1.0 + np.exp(l2 - l1))).astype(np.float32)
    g2 = (1.0 - g1).astype(np.float32)

    # ---- dispatch: gather per-expert tokens, pre-tile (fp16) ----
    in_maps = []
    idx_per_e = []
    gv_per_e = []
    for e in range(E):
        sel1 = np.nonzero(i1 == e)[0]
        sel2 = np.nonzero(i2 == e)[0]
        idx = np.concatenate([sel1, sel2])
        gv = np.concatenate([g1[sel1], g2[sel2]])
        idx_per_e.append(idx)
        gv_per_e.append(gv)
        dev = min(idx.shape[0], C)

        xg = np.zeros((C, D), np.float32)
        xg[:dev] = xf[idx[:dev]]
        # [c, q, p, r, t]: xgt[c, q, p, r*512+t] = Xg[c*512+t, (4q+r)*128+p]
        xgt = np.ascontiguousarray(
            xg.T.reshape(2, 4, P, NCH, 512).transpose(3, 0, 2, 1, 4).reshape(
                NCH, 2, P, 4 * 512)).astype(np.float16)
        w1t = np.ascontiguousarray(
            w1[e].reshape(DC, P, HC, P).transpose(2, 1, 0, 3).reshape(
                HC, P, DC * P)).astype(np.float16)
        w2t = np.ascontiguousarray(
            w2[e].reshape(HC, P, DC, P).transpose(2, 1, 0, 3).reshape(
                DC, P, HC * P)).astype(np.float16)
        b1t = np.ascontiguousarray(b1[e].reshape(HC, P).T)
        in_maps.append({"xgt": xgt, "w1t": w1t, "w2t": w2t, "b1t": b1t})

    nc = _get_nc()
    res = bass_utils.run_bass_kernel_spmd(
        nc, in_maps, core_ids=list(range(N_CORES)), trace=TRACE)
    global LAST_RESULTS
    LAST_RESULTS = res

    # ---- combine (host): gate scale + top-2 sum; overflow tokens beyond
    # device capacity get their exact fp32 FFN here (~1% of pairs) ----
    out = np.zeros((N, D), np.float32)
    for e in range(E):
        idx = idx_per_e[e]
        gv = gv_per_e[e]
        dev = min(idx.shape[0], C)
        y = res.results[e]["ygt"][:, :dev].astype(np.float32).T  # [dev, D]
        out[idx[:dev]] += gv[:dev, None] * y
        if idx.shape[0] > C:
            xs = xf[idx[C:]]                                     # [S, D]
            hs = xs @ w1[e] + b1[e].reshape(1, D_HID)
            hs = 0.5 * hs * (1.0 + _erf(hs / np.sqrt(2.0)))
            ys = (hs @ w2[e]).astype(np.float32)
            out[idx[C:]] += gv[C:, None] * ys

    if np.any(b2):
        gate_full = np.zeros((N, E), np.float32)
        gate_full[np.arange(N), i1] = g1
        gate_full[np.arange(N), i2] = g2
        out += gate_full @ b2.reshape(E, D)

    return out.reshape(B, T, D)



# revision 4
# speedup vs baseline: 1.0720x; 1.0720x over previous
"""MoE FFN (E=8 experts, top-2) — expert-parallel Bass/Tile kernel for 8 TRN2 cores.

Strategy:
  - Host computes the (tiny) router: logits = x @ gate_w.T, top-2 per token,
    renormalized weights.  Token n is dispatched to cores e1(n), e2(n)
    (expert-parallel: core e holds expert e's weights).
  - Device capacity C=1024 tokens per expert (= the perfectly balanced
    N*K/E share).  Overflow tokens beyond 1024 per expert (~1.3% of
    pairs, pure load imbalance) are computed exactly on the host during
    combine.
  - Per-expert tokens are sorted by gate weight DESCENDING.  The first
    640 (large gates) take the fp16 path; the last 384 (gate <= ~0.46)
    use fp8e4m3 DoubleRow matmuls for mm2 (2x PE rate, measured 216ns
    for K=256/M=128/N=512 vs fp16's 216ns at half the MACs).  Their
    error (~3.4% RMS on y) is attenuated by the small gate: simulated
    end-to-end rel err 1.5e-2 vs the 2e-2 budget.
  - mm1 (fp16 for ALL tokens): hT[hc, c] = gelu(w1.T @ xgT + b1); 64
    chains of 8 matmuls.  Chunk c1's gelu eviction splits: cols 0:128
    (tokens 512:640) -> fp16 ht; cols 128:512 (tokens 640:1024) -> fp8
    ht in DoubleRow pair layout [p, 2, 384].
  - mm2 per dc: fp16 chains for token chunks [0:512) (N=512) and
    [512:640) (N=128) interleaved so each w2 tile's two LDWEIGHTS
    (2x97ns) hide under 213+53ns of matmul; then one fp8 DR chain of 16
    matmuls (K=256 each) over [640:1024) (N=384).  w2 ships both as
    fp16 tiles and as x64-scaled fp8 DR tiles (+4MB DMA, ~free at the
    measured ~430GB/s aggregate DMA bandwidth); the 1/64 is folded into
    the PSUM eviction scale.
  - 24 warmup matmuls on a memset tile start the PE at ~4us (vs ~10.5us
    first-DMA-ready), ramping the p-state clock while the head DMAs
    stream.
  - Engine assignment: PE = matmuls, scalar(Act) = gelu + w2/b1 DMA
    ring, sync(SP) = xg/w1/output DMA ring, vector(DVE) = PSUM
    evictions + warmup memset.
"""

import re

import numpy as np
import ml_dtypes

import bass_rust
import concourse.bass as bass
import concourse.mybir as mybir
import concourse.tile as tile
from concourse import bacc, bass_utils

P = 128
D_MODEL = 1024
D_HID = 4096
E = 8
TOP_K = 2
N_CORES = 8

DC = D_MODEL // P          # 8 d-chunks
HC = D_HID // P            # 32 h-chunks
HC2 = HC // 2              # 16 DoubleRow K-chunks (256 rows each)
C = 1024                   # device per-expert token capacity (rest -> host)
NCH = C // 512             # 2 token chunks of 512 (one PSUM bank each)
SPLIT = 640                # tokens [0:SPLIT) fp16 path, [SPLIT:C) fp8-mm2
NF8 = C - SPLIT            # 384 fp8 tokens
SC = 64.0                  # w2 fp8 pre-scale (lifts values out of denormals)
WARMUP = 24                # PE warmup matmuls (cover DMA head + pstate ramp)

F32 = mybir.dt.float32
F16 = mybir.dt.float16
F8 = mybir.dt.float8e4
MM_DT = F16
DR = mybir.MatmulPerfMode.DoubleRow
GELU = mybir.ActivationFunctionType.Gelu

W2_BUFS = 3                # w2 stream depth (tiles of [128, 4096])

E4M3 = ml_dtypes.float8_e4m3

_tail_patched = False


def _patch_light_tail():
    """Replace Tile's end-of-context machinery (multi-wait drain + two
    all-engine EVSEM barriers + semaphore range-clears, ~10us on HW) with
    single-wait drains on the sync engine covering every logical proc's final
    tick.  The NEFF is executed once per load in this flow, so semaphores
    need not be recycled."""
    global _tail_patched
    if _tail_patched:
        return
    _tail_patched = True

    def _drain_and_barrier(self, tick_clock, wait_clock):
        gc = tick_clock.global_clock
        ticks = eval(re.match(r"VectorClock\((.*)\)", repr(gc)).group(1))
        n = len(ticks)
        for i, v in enumerate(ticks):
            if v > 0:
                vc = bass_rust.VectorClock(
                    [v if j == i else 0 for j in range(n)])
                w = self.nc.sync.drain()
                wait_clock.add_sem_waits(
                    w.ins,
                    bass_rust.ScopedClock({None: vc}),
                    bass_rust.ScopedClock({}),
                )
        popped = self.nc._tile_sem_poison_stack.pop()
        assert popped is self._sem_poison
    tile.TileContext._drain_and_barrier = _drain_and_barrier


def build_nc():
    _patch_light_tail()
    nc = bacc.Bacc("TRN2", target_bir_lowering=False, debug=False,
                   num_devices=N_CORES)

    # Inputs, pre-tiled on host into consumption order (contiguous DMAs):
    #   xgt [NCH, 2, P, 2048]    xgt[c, q, p, r*512+t] = Xg[c*512+t, (4q+r)*128+p]
    #   w1t [HC, P, DC*P]        w1t[hc, p, dc*128+j] = w1[dc*128+p, hc*128+j]
    #   w2t [DC, P, HC*P]        w2t[dc, p, hc*128+j] = w2[hc*128+p, dc*128+j]
    #   w28t [DC, P, HC2, 2, P]  w28t[dc,p,k,i,j] = e4m3(64*w2[(2k+i)*128+p, dc*128+j])
    #   b1t [P, HC]              b1t[p, hc] = b1[hc*128+p]
    # Output:
    #   ygt [D, C]               ygt[d, n] = Y[n, d]   (pre-gate, fp16;
    #                            cols [SPLIT:) carry the exact value — the
    #                            x64 w2 scale is folded out at eviction)
    xgt = nc.dram_tensor("xgt", [NCH, 2, P, 4 * 512], MM_DT, kind="ExternalInput")
    w1t = nc.dram_tensor("w1t", [HC, P, DC * P], MM_DT, kind="ExternalInput")
    w2t = nc.dram_tensor("w2t", [DC, P, HC * P], MM_DT, kind="ExternalInput")
    w28t = nc.dram_tensor("w28t", [DC, P, HC2, 2, P], F8, kind="ExternalInput")
    b1t = nc.dram_tensor("b1t", [P, HC], F32, kind="ExternalInput")
    ygt = nc.dram_tensor("ygt", [D_MODEL, C], MM_DT, kind="ExternalOutput")

    with tile.TileContext(nc) as tc:
        with (
            tc.tile_pool(name="const", bufs=1) as const,
            tc.tile_pool(name="xg", bufs=1) as xg_pool,
            tc.tile_pool(name="w1", bufs=1) as w1_pool,
            tc.tile_pool(name="w2", bufs=W2_BUFS) as w2_pool,
            tc.tile_pool(name="w28", bufs=W2_BUFS) as w28_pool,
            tc.tile_pool(name="ht", bufs=1) as ht_pool,
            tc.tile_pool(name="yo", bufs=2) as yo_pool,
            # PSUM budget (8 banks of [128, 2KB]):
            #   ps1 3 (mm1 chains) + psf 2 (warmup + fp8 chain)
            #   + psa 2 (N=512 chains) + psb 1 (N=128 chains) = 8
            tc.tile_pool(name="ps1", bufs=3, space="PSUM") as ps1,
            tc.tile_pool(name="psf", bufs=1, space="PSUM") as psf,
            tc.tile_pool(name="psa", bufs=2, space="PSUM") as psa,
            tc.tile_pool(name="psb", bufs=1, space="PSUM") as psb,
        ):
            b1_sb = const.tile([P, HC], F32, name="b1sb")
            warm = const.tile([P, 512], F16, name="warm")

            # PE warmup: zeros matmuls with no DMA deps start the PE at
            # ~4us (engine-init limited) instead of ~10.5us (DMA-head
            # limited) and finish the p-state ramp before real work.
            nc.vector.memset(warm[:], 0.0)
            for _ in range(WARMUP):
                psw = psf.tile([P, 512], F32, name="psw")
                nc.tensor.matmul(psw[:], lhsT=warm[:, :P], rhs=warm[:],
                                 start=True, stop=True)

            # Head DMAs: what pass A (token chunk c0) needs — w1#0 in two
            # halves plus the two 512KB xg-c0 quartets — split across the
            # two HWDGE rings in consumption order.
            xg_sb = {}
            w1_sb = {}

            def w1_load(hc, eng):
                t = w1_pool.tile([P, DC * P], MM_DT, name=f"w1_{hc}")
                eng.dma_start(out=t[:], in_=w1t[hc, :, :])
                w1_sb[hc] = t

            def xg_load(c, q, eng):
                t = xg_pool.tile([P, 4 * 512], MM_DT, name=f"xg{c}_{q}")
                eng.dma_start(out=t[:], in_=xgt[c, q, :, :])
                xg_sb[(c, q)] = t

            # First pieces cut small (subtile deps) so the first real
            # matmul waits on just 32KB of w1 + 128KB of xg.
            w1_sb[0] = w1_pool.tile([P, DC * P], MM_DT, name="w1_0")
            t = xg_pool.tile([P, 4 * 512], MM_DT, name="xg0_0")
            nc.scalar.dma_start(out=w1_sb[0][:, :P], in_=w1t[0, :, :P])
            nc.sync.dma_start(out=t[:, :512], in_=xgt[0, 0, :, :512])
            nc.scalar.dma_start(out=w1_sb[0][:, P:], in_=w1t[0, :, P:])
            nc.sync.dma_start(out=t[:, 512:], in_=xgt[0, 0, :, 512:])
            xg_sb[(0, 0)] = t
            nc.scalar.dma_start(out=b1_sb[:], in_=b1t[:, :])
            xg_load(0, 1, nc.sync)
            w1_load(2, nc.scalar)
            w1_load(1, nc.sync)

            # ---- mm1: hT[hc, c] = gelu(w1.T @ xgT + b1), fp16 for all ----
            ht_c0 = {}                 # [128, 512] fp16  (tokens 0:512)
            ht_c1b = {}                # [128, 128] fp16  (tokens 512:640)
            ht8 = {}                   # [128, 2, 384] fp8 (tokens 640:1024)
            w2_sb = {}
            w28_sb = {}

            def w28_load(k, eng):
                t = w28_pool.tile([P, HC2, 2, P], F8, name="w28sb")
                eng.dma_start(out=t[:], in_=w28t[k, :, :, :, :])
                w28_sb[k] = t

            for c in range(NCH):
                for hc in range(HC):
                    if c == 0:
                        if hc + 3 < HC:
                            nhc = hc + 3
                            w1_load(nhc, nc.sync if nhc % 2 == 1
                                    else nc.scalar)
                        if hc == 4:
                            xg_load(1, 0, nc.sync)
                        if hc == 6:
                            xg_load(1, 1, nc.scalar)
                        if hc in (14, 19, 24):
                            k = {14: 0, 19: 1, 24: 2}[hc]
                            t = w2_pool.tile([P, HC * P], MM_DT,
                                             name="w2sb")
                            nc.scalar.dma_start(out=t[:], in_=w2t[k, :, :])
                            w2_sb[k] = t
                    else:
                        if hc in (8, 16, 24):
                            k = {8: 0, 16: 1, 24: 2}[hc]
                            w28_load(k, nc.scalar if hc != 16 else nc.sync)
                    ps = ps1.tile([P, 512], F32, name="ps1")
                    for dc in range(DC):
                        nc.tensor.matmul(
                            ps[:],
                            lhsT=w1_sb[hc][:, dc * P:(dc + 1) * P],
                            rhs=xg_sb[(c, dc // 4)][
                                :, (dc % 4) * 512:(dc % 4 + 1) * 512],
                            start=(dc == 0),
                            stop=(dc == DC - 1),
                        )
                    bias = b1_sb[:, hc:hc + 1]
                    if c == 0:
                        ht = ht_pool.tile([P, 512], MM_DT, name=f"ht{hc}")
                        nc.scalar.activation(ht[:], ps[:], GELU, bias=bias)
                        ht_c0[hc] = ht
                    else:
                        if hc % 2 == 0:
                            ht8[hc // 2] = ht_pool.tile(
                                [P, 2, NF8], F8, name=f"ht8_{hc // 2}")
                        htb = ht_pool.tile([P, P], MM_DT, name=f"htb{hc}")
                        nc.scalar.activation(htb[:], ps[:, :P], GELU,
                                             bias=bias)
                        ht_c1b[hc] = htb
                        nc.scalar.activation(ht8[hc // 2][:, hc % 2, :],
                                             ps[:, P:512], GELU, bias=bias)

            # ---- mm2: Y^T[dc] = sum_hc w2tile.T @ hT[hc] ----
            # fp16 N=512 + N=128 chains interleaved (shared lhsT), then
            # the fp8 DoubleRow N=384 chain (16 K=256 matmuls).
            for dc in range(DC):
                if dc + W2_BUFS < DC:
                    ndc = dc + W2_BUFS
                    t = w2_pool.tile([P, HC * P], MM_DT, name="w2sb")
                    nc.scalar.dma_start(out=t[:], in_=w2t[ndc, :, :])
                    w2_sb[ndc] = t
                    w28_load(ndc, nc.sync)
                ps_a = psa.tile([P, 512], F32, name="ps2a")
                ps_b = psb.tile([P, 512], F32, name="ps2b")
                for hc in range(HC):
                    l = w2_sb[dc][:, hc * P:(hc + 1) * P]
                    nc.tensor.matmul(ps_a[:], lhsT=l, rhs=ht_c0[hc][:],
                                     start=(hc == 0), stop=(hc == HC - 1))
                    nc.tensor.matmul(ps_b[:, :P], lhsT=l,
                                     rhs=ht_c1b[hc][:],
                                     start=(hc == 0), stop=(hc == HC - 1))
                yo_a = yo_pool.tile([P, 512], MM_DT, name="yoa")
                nc.vector.tensor_scalar_mul(yo_a[:], ps_a[:], 1.0)
                nc.sync.dma_start(
                    out=ygt[dc * P:(dc + 1) * P, 0:512], in_=yo_a[:])
                yo_b = yo_pool.tile([P, 512], MM_DT, name="yob")
                nc.vector.tensor_scalar_mul(yo_b[:, :P], ps_b[:, :P], 1.0)
                nc.scalar.dma_start(
                    out=ygt[dc * P:(dc + 1) * P, 512:SPLIT],
                    in_=yo_b[:, :P])
                ps_f = psf.tile([P, 512], F32, name="ps2f")
                for k in range(HC2):
                    nc.tensor.matmul(
                        ps_f[:, :NF8],
                        lhsT=w28_sb[dc][:, k, :, :],
                        rhs=ht8[k][:, :, :],
                        start=(k == 0), stop=(k == HC2 - 1),
                        perf_mode=DR)
                yo_f = yo_pool.tile([P, 512], MM_DT, name="yof")
                nc.vector.tensor_scalar_mul(yo_f[:, :NF8], ps_f[:, :NF8],
                                            1.0 / SC)
                nc.sync.dma_start(
                    out=ygt[dc * P:(dc + 1) * P, SPLIT:C],
                    in_=yo_f[:, :NF8])
    nc.compile()
    return nc


_NC_CACHE = None
TRACE = False
LAST_RESULTS = None


def _get_nc():
    global _NC_CACHE
    if _NC_CACHE is None:
        _NC_CACHE = build_nc()
    return _NC_CACHE


def _erf(x):
    try:
        from scipy.special import erf
        return erf(x)
    except ImportError:
        import math
        return np.frompyfunc(math.erf, 1, 1)(x).astype(np.float64)


def kernel(x, gate_w, w1, b1, w2, b2):
    x = np.asarray(x, dtype=np.float32)
    gate_w = np.asarray(gate_w, dtype=np.float32)
    w1 = np.asarray(w1, dtype=np.float32)
    b1 = np.asarray(b1, dtype=np.float32)
    w2 = np.asarray(w2, dtype=np.float32)
    b2 = np.asarray(b2, dtype=np.float32)

    B, T, D = x.shape
    N = B * T
    xf = x.reshape(N, D)

    # ---- router (host; 0.05% of model FLOPs — the sharding decision) ----
    logits = xf @ gate_w.T                           # [N, E]
    order = np.argsort(-logits, axis=1, kind="stable")
    i1, i2 = order[:, 0], order[:, 1]
    l1 = logits[np.arange(N), i1].astype(np.float64)
    l2 = logits[np.arange(N), i2].astype(np.float64)
    g1 = (1.0 / (1.0 + np.exp(l2 - l1))).astype(np.float32)
    g2 = (1.0 - g1).astype(np.float32)

    # ---- dispatch: gather per-expert tokens sorted by gate desc ----
    in_maps = []
    idx_per_e = []
    gv_per_e = []
    for e in range(E):
        sel1 = np.nonzero(i1 == e)[0]
        sel2 = np.nonzero(i2 == e)[0]
        idx = np.concatenate([sel1, sel2])
        gv = np.concatenate([g1[sel1], g2[sel2]])
        o = np.argsort(-gv, kind="stable")
        idx, gv = idx[o], gv[o]
        idx_per_e.append(idx)
        gv_per_e.append(gv)
        dev = min(idx.shape[0], C)

        xg = np.zeros((C, D), np.float32)
        xg[:dev] = xf[idx[:dev]]
        # [c, q, p, r, t]: xgt[c, q, p, r*512+t] = Xg[c*512+t, (4q+r)*128+p]
        xgt = np.ascontiguousarray(
            xg.T.reshape(2, 4, P, NCH, 512).transpose(3, 0, 2, 1, 4).reshape(
                NCH, 2, P, 4 * 512)).astype(np.float16)
        w1t = np.ascontiguousarray(
            w1[e].reshape(DC, P, HC, P).transpose(2, 1, 0, 3).reshape(
                HC, P, DC * P)).astype(np.float16)
        w2t = np.ascontiguousarray(
            w2[e].reshape(HC, P, DC, P).transpose(2, 1, 0, 3).reshape(
                DC, P, HC * P)).astype(np.float16)
        # DoubleRow fp8 w2: [dc, p, k, i, m] = e4m3(64*w2[(2k+i)*128+p, dc*128+m])
        w28t = np.ascontiguousarray(
            (w2[e] * SC).reshape(HC2, 2, P, DC, P).transpose(3, 2, 0, 1, 4)
        ).astype(E4M3)
        b1t = np.ascontiguousarray(b1[e].reshape(HC, P).T)
        in_maps.append({"xgt": xgt, "w1t": w1t, "w2t": w2t, "w28t": w28t,
                        "b1t": b1t})

    nc = _get_nc()
    res = bass_utils.run_bass_kernel_spmd(
        nc, in_maps, core_ids=list(range(N_CORES)), trace=TRACE)
    global LAST_RESULTS
    LAST_RESULTS = res

    # ---- combine (host): gate scale + top-2 sum; overflow tokens beyond
    # device capacity get their exact fp32 FFN here (~1% of pairs) ----
    out = np.zeros((N, D), np.float32)
    for e in range(E):
        idx = idx_per_e[e]
        gv = gv_per_e[e]
        dev = min(idx.shape[0], C)
        y = res.results[e]["ygt"][:, :dev].astype(np.float32).T  # [dev, D]
        out[idx[:dev]] += gv[:dev, None] * y
        if idx.shape[0] > C:
            xs = xf[idx[C:]]                                     # [S, D]
            hs = xs @ w1[e] + b1[e].reshape(1, D_HID)
            hs = 0.5 * hs * (1.0 + _erf(hs / np.sqrt(2.0)))
            ys = (hs @ w2[e]).astype(np.float32)
            out[idx[C:]] += gv[C:, None] * ys

    if np.any(b2):
        gate_full = np.zeros((N, E), np.float32)
        gate_full[np.arange(N), i1] = g1
        gate_full[np.arange(N), i2] = g2
        out += gate_full @ b2.reshape(E, D)

    return out.reshape(B, T, D)


# revision 6
# speedup vs baseline: 1.2700x; 1.1847x over previous
"""MoE FFN (E=8 experts, top-2) — expert-parallel Bass/Tile kernel for 8 TRN2 cores.

Strategy:
  - Host computes the (tiny) router: logits = x @ gate_w.T, top-2 per token,
    renormalized weights.  Token n is dispatched to cores e1(n), e2(n)
    (expert-parallel: core e holds expert e's weights).
  - Device capacity C=1024 tokens per expert (= the perfectly balanced
    N*K/E share).  Overflow tokens beyond 1024 per expert (~1.3% of
    pairs, pure load imbalance) are computed exactly on the host during
    combine.
  - Per-expert tokens are sorted by gate weight DESCENDING.  The first
    640 (large gates) take the fp16 path; the last 384 (gate <= ~0.46)
    use fp8e4m3 DoubleRow matmuls for mm2 (2x PE rate, measured 216ns
    for K=256/M=128/N=512 vs fp16's 216ns at half the MACs).  Their
    error (~3.4% RMS on y) is attenuated by the small gate: simulated
    end-to-end rel err 1.5e-2 vs the 2e-2 budget.
  - mm1 (fp16 for ALL tokens): hT[hc, c] = gelu(w1.T @ xgT + b1); 64
    chains of 8 matmuls.  Chunk c1's gelu eviction splits: cols 0:128
    (tokens 512:640) -> fp16 ht; cols 128:512 (tokens 640:1024) -> fp8
    ht in DoubleRow pair layout [p, 2, 384].
  - mm2 per dc: fp16 chains for token chunks [0:512) (N=512) and
    [512:640) (N=128) interleaved so each w2 tile's two LDWEIGHTS
    (2x97ns) hide under 213+53ns of matmul; then one fp8 DR chain of 16
    matmuls (K=256 each) over [640:1024) (N=384).  w2 ships both as
    fp16 tiles and as x64-scaled fp8 DR tiles (+4MB DMA, ~free at the
    measured ~430GB/s aggregate DMA bandwidth); the 1/64 is folded into
    the PSUM eviction scale.
  - 24 warmup matmuls on a memset tile start the PE at ~4us (vs ~10.5us
    first-DMA-ready), ramping the p-state clock while the head DMAs
    stream.
  - Engine assignment: PE = matmuls, scalar(Act) = gelu + w2/b1 DMA
    ring, sync(SP) = xg/w1/output DMA ring, vector(DVE) = PSUM
    evictions + warmup memset.
"""

import re

import numpy as np
import ml_dtypes

import bass_rust
import concourse.bass as bass
import concourse.mybir as mybir
import concourse.tile as tile
from concourse import bacc, bass_utils

P = 128
D_MODEL = 1024
D_HID = 4096
E = 8
TOP_K = 2
N_CORES = 8

DC = D_MODEL // P          # 8 d-chunks
HC = D_HID // P            # 32 h-chunks
HC2 = HC // 2              # 16 DoubleRow K-chunks (256 rows each)
C = 1024                   # device per-expert token capacity (rest -> host)
NCH = C // 512             # 2 token chunks of 512 (one PSUM bank each)
SPLIT = 640                # tokens [0:SPLIT) fp16 path, [SPLIT:C) fp8-mm2
NF8 = C - SPLIT            # 384 fp8 tokens
SC = 64.0                  # w2 fp8 pre-scale (lifts values out of denormals)
WARMUP = 24                # PE warmup matmuls (cover DMA head + pstate ramp)

F32 = mybir.dt.float32
F16 = mybir.dt.float16
F8 = mybir.dt.float8e4
MM_DT = F16
DR = mybir.MatmulPerfMode.DoubleRow
GELU = mybir.ActivationFunctionType.Gelu

W2_BUFS = 3                # w2 stream depth (tiles of [128, 4096])

E4M3 = ml_dtypes.float8_e4m3

_tail_patched = False


def _patch_light_tail():
    """Replace Tile's end-of-context machinery (multi-wait drain + two
    all-engine EVSEM barriers + semaphore range-clears, ~10us on HW) with
    single-wait drains on the sync engine covering every logical proc's final
    tick.  The NEFF is executed once per load in this flow, so semaphores
    need not be recycled."""
    global _tail_patched
    if _tail_patched:
        return
    _tail_patched = True

    def _drain_and_barrier(self, tick_clock, wait_clock):
        gc = tick_clock.global_clock
        ticks = eval(re.match(r"VectorClock\((.*)\)", repr(gc)).group(1))
        n = len(ticks)
        for i, v in enumerate(ticks):
            if v > 0:
                vc = bass_rust.VectorClock(
                    [v if j == i else 0 for j in range(n)])
                w = self.nc.sync.drain()
                wait_clock.add_sem_waits(
                    w.ins,
                    bass_rust.ScopedClock({None: vc}),
                    bass_rust.ScopedClock({}),
                )
        popped = self.nc._tile_sem_poison_stack.pop()
        assert popped is self._sem_poison
    tile.TileContext._drain_and_barrier = _drain_and_barrier


def build_nc():
    _patch_light_tail()
    nc = bacc.Bacc("TRN2", target_bir_lowering=False, debug=False,
                   num_devices=N_CORES)

    # Inputs, pre-tiled on host into consumption order (contiguous DMAs):
    #   xgt [NCH, 2, P, 2048]    xgt[c, q, p, r*512+t] = Xg[c*512+t, (4q+r)*128+p]
    #   w1t [HC, P, DC*P]        w1t[hc, p, dc*128+j] = w1[dc*128+p, hc*128+j]
    #   w2t [DC, P, HC*P]        w2t[dc, p, hc*128+j] = w2[hc*128+p, dc*128+j]
    #   w28t [DC, P, HC2, 2, P]  w28t[dc,p,k,i,j] = e4m3(64*w2[(2k+i)*128+p, dc*128+j])
    #   b1t [P, HC]              b1t[p, hc] = b1[hc*128+p]
    # Output:
    #   ygt [D, C]               ygt[d, n] = Y[n, d]   (pre-gate, fp16;
    #                            cols [SPLIT:) carry the exact value — the
    #                            x64 w2 scale is folded out at eviction)
    xgt = nc.dram_tensor("xgt", [NCH, 2, P, 4 * 512], MM_DT, kind="ExternalInput")
    w1t = nc.dram_tensor("w1t", [HC, P, DC * P], MM_DT, kind="ExternalInput")
    w2t = nc.dram_tensor("w2t", [DC, P, HC * P], MM_DT, kind="ExternalInput")
    w28t = nc.dram_tensor("w28t", [DC, P, HC2, 2, P], F8, kind="ExternalInput")
    b1t = nc.dram_tensor("b1t", [P, HC], F32, kind="ExternalInput")
    ygt = nc.dram_tensor("ygt", [D_MODEL, C], MM_DT, kind="ExternalOutput")

    with tile.TileContext(nc) as tc:
        with (
            tc.tile_pool(name="const", bufs=1) as const,
            tc.tile_pool(name="xg", bufs=1) as xg_pool,
            tc.tile_pool(name="w1", bufs=1) as w1_pool,
            tc.tile_pool(name="w2", bufs=W2_BUFS) as w2_pool,
            tc.tile_pool(name="w28", bufs=W2_BUFS) as w28_pool,
            tc.tile_pool(name="ht", bufs=1) as ht_pool,
            tc.tile_pool(name="yo", bufs=2) as yo_pool,
            # PSUM budget (8 banks of [128, 2KB]):
            #   ps1 3 (mm1 chains) + psf 2 (warmup + fp8 chain)
            #   + psa 2 (N=512 chains) + psb 1 (N=128 chains) = 8
            tc.tile_pool(name="ps1", bufs=3, space="PSUM") as ps1,
            tc.tile_pool(name="psf", bufs=1, space="PSUM") as psf,
            tc.tile_pool(name="psa", bufs=2, space="PSUM") as psa,
            tc.tile_pool(name="psb", bufs=1, space="PSUM") as psb,
        ):
            b1_sb = const.tile([P, HC], F32, name="b1sb")
            warm = const.tile([P, 512], F16, name="warm")

            # PE warmup: matmuls gated only on a DVE memset start the PE
            # early (vs ~10.5us DMA-head limited) and finish the p-state
            # ramp before real work.  The warm PSUM tile is never read.
            nc.vector.memset(warm[:], 0.0)
            for _ in range(WARMUP):
                psw = psf.tile([P, 512], F32, name="psw")
                nc.tensor.matmul(psw[:], lhsT=warm[:, :P], rhs=warm[:],
                                 start=True, stop=True)

            # Head DMAs: what pass A (token chunk c0) needs — w1#0 in two
            # halves plus the two 512KB xg-c0 quartets — split across the
            # two HWDGE rings in consumption order.
            xg_sb = {}
            w1_sb = {}

            def w1_load(hc, eng):
                t = w1_pool.tile([P, DC * P], MM_DT, name=f"w1_{hc}")
                eng.dma_start(out=t[:], in_=w1t[hc, :, :])
                w1_sb[hc] = t

            def xg_load(c, q, eng):
                t = xg_pool.tile([P, 4 * 512], MM_DT, name=f"xg{c}_{q}")
                eng.dma_start(out=t[:], in_=xgt[c, q, :, :])
                xg_sb[(c, q)] = t

            # First pieces cut small (subtile deps) so the first real
            # matmul waits on just 32KB of w1 + 128KB of xg.
            w1_sb[0] = w1_pool.tile([P, DC * P], MM_DT, name="w1_0")
            t = xg_pool.tile([P, 4 * 512], MM_DT, name="xg0_0")
            nc.scalar.dma_start(out=w1_sb[0][:, :P], in_=w1t[0, :, :P])
            nc.sync.dma_start(out=t[:, :512], in_=xgt[0, 0, :, :512])
            nc.scalar.dma_start(out=w1_sb[0][:, P:], in_=w1t[0, :, P:])
            nc.sync.dma_start(out=t[:, 512:], in_=xgt[0, 0, :, 512:])
            xg_sb[(0, 0)] = t
            nc.scalar.dma_start(out=b1_sb[:], in_=b1t[:, :])
            xg_load(0, 1, nc.sync)
            w1_load(2, nc.scalar)
            w1_load(1, nc.sync)
            w1_load(4, nc.scalar)
            w1_load(3, nc.sync)

            # ---- mm1: hT[hc, c] = gelu(w1.T @ xgT + b1), fp16 for all ----
            ht_c0 = {}                 # [128, 512] fp16  (tokens 0:512)
            ht_c1b = {}                # [128, 128] fp16  (tokens 512:640)
            ht8 = {}                   # [128, 2, 384] fp8 (tokens 640:1024)
            w2_sb = {}
            w28_sb = {}

            def w28_load(k, eng):
                t = w28_pool.tile([P, HC2, 2, P], F8, name="w28sb")
                eng.dma_start(out=t[:], in_=w28t[k, :, :, :, :])
                w28_sb[k] = t

            for c in range(NCH):
                for hc in range(HC):
                    if c == 0:
                        if hc + 5 < HC:
                            nhc = hc + 5
                            w1_load(nhc, nc.sync if nhc % 2 == 1
                                    else nc.scalar)
                        if hc == 4:
                            xg_load(1, 0, nc.sync)
                        if hc == 6:
                            xg_load(1, 1, nc.scalar)
                        if hc in (14, 19, 24):
                            k = {14: 0, 19: 1, 24: 2}[hc]
                            t = w2_pool.tile([P, HC * P], MM_DT,
                                             name="w2sb")
                            nc.scalar.dma_start(out=t[:], in_=w2t[k, :, :])
                            w2_sb[k] = t
                    else:
                        if hc in (8, 16, 24):
                            k = {8: 0, 16: 1, 24: 2}[hc]
                            w28_load(k, nc.scalar if hc != 16 else nc.sync)
                    ps = ps1.tile([P, 512], F32, name="ps1")
                    for dc in range(DC):
                        nc.tensor.matmul(
                            ps[:],
                            lhsT=w1_sb[hc][:, dc * P:(dc + 1) * P],
                            rhs=xg_sb[(c, dc // 4)][
                                :, (dc % 4) * 512:(dc % 4 + 1) * 512],
                            start=(dc == 0),
                            stop=(dc == DC - 1),
                        )
                    bias = b1_sb[:, hc:hc + 1]
                    if c == 0:
                        ht = ht_pool.tile([P, 512], MM_DT, name=f"ht{hc}")
                        nc.scalar.activation(ht[:], ps[:], GELU, bias=bias)
                        ht_c0[hc] = ht
                    else:
                        if hc % 2 == 0:
                            ht8[hc // 2] = ht_pool.tile(
                                [P, 2, NF8], F8, name=f"ht8_{hc // 2}")
                        htb = ht_pool.tile([P, P], MM_DT, name=f"htb{hc}")
                        nc.scalar.activation(htb[:], ps[:, :P], GELU,
                                             bias=bias)
                        ht_c1b[hc] = htb
                        nc.scalar.activation(ht8[hc // 2][:, hc % 2, :],
                                             ps[:, P:512], GELU, bias=bias)

            # ---- mm2: Y^T[dc] = sum_hc w2tile.T @ hT[hc] ----
            # fp16 N=512 + N=128 chains interleaved (shared lhsT), then
            # the fp8 DoubleRow N=384 chain (16 K=256 matmuls).
            for dc in range(DC):
                if dc + W2_BUFS < DC:
                    ndc = dc + W2_BUFS
                    t = w2_pool.tile([P, HC * P], MM_DT, name="w2sb")
                    nc.scalar.dma_start(out=t[:], in_=w2t[ndc, :, :])
                    w2_sb[ndc] = t
                    w28_load(ndc, nc.sync)
                def fp16_pair(dc):
                    ps_a = psa.tile([P, 512], F32, name="ps2a")
                    ps_b = psb.tile([P, 512], F32, name="ps2b")
                    for hc in range(HC):
                        l = w2_sb[dc][:, hc * P:(hc + 1) * P]
                        nc.tensor.matmul(ps_a[:], lhsT=l, rhs=ht_c0[hc][:],
                                         start=(hc == 0),
                                         stop=(hc == HC - 1))
                        nc.tensor.matmul(ps_b[:, :P], lhsT=l,
                                         rhs=ht_c1b[hc][:],
                                         start=(hc == 0),
                                         stop=(hc == HC - 1))
                    yo_a = yo_pool.tile([P, 512], MM_DT, name="yoa")
                    nc.vector.tensor_scalar_mul(yo_a[:], ps_a[:], 1.0)
                    nc.sync.dma_start(
                        out=ygt[dc * P:(dc + 1) * P, 0:512], in_=yo_a[:])
                    yo_b = yo_pool.tile([P, 512], MM_DT, name="yob")
                    nc.vector.tensor_scalar_mul(yo_b[:, :P], ps_b[:, :P],
                                                1.0)
                    nc.scalar.dma_start(
                        out=ygt[dc * P:(dc + 1) * P, 512:SPLIT],
                        in_=yo_b[:, :P])

                def fp8_chain(dc):
                    ps_f = psf.tile([P, 512], F32, name="ps2f")
                    for k in range(HC2):
                        nc.tensor.matmul(
                            ps_f[:, :NF8],
                            lhsT=w28_sb[dc][:, k, :, :],
                            rhs=ht8[k][:, :, :],
                            start=(k == 0), stop=(k == HC2 - 1),
                            perf_mode=DR)
                    yo_f = yo_pool.tile([P, 512], MM_DT, name="yof")
                    nc.vector.tensor_scalar_mul(yo_f[:, :NF8],
                                                ps_f[:, :NF8], 1.0 / SC)
                    nc.sync.dma_start(
                        out=ygt[dc * P:(dc + 1) * P, SPLIT:C],
                        in_=yo_f[:, :NF8])

                # last dc ends on the small fp16 eviction (short tail)
                if dc == DC - 1:
                    fp8_chain(dc)
                    fp16_pair(dc)
                else:
                    fp16_pair(dc)
                    fp8_chain(dc)
    nc.compile()
    return nc


_NC_CACHE = None
TRACE = False
LAST_RESULTS = None


def _get_nc():
    global _NC_CACHE
    if _NC_CACHE is None:
        _NC_CACHE = build_nc()
    return _NC_CACHE


def _erf(x):
    try:
        from scipy.special import erf
        return erf(x)
    except ImportError:
        import math
        return np.frompyfunc(math.erf, 1, 1)(x).astype(np.float64)


def kernel(x, gate_w, w1, b1, w2, b2):
    x = np.asarray(x, dtype=np.float32)
    gate_w = np.asarray(gate_w, dtype=np.float32)
    w1 = np.asarray(w1, dtype=np.float32)
    b1 = np.asarray(b1, dtype=np.float32)
    w2 = np.asarray(w2, dtype=np.float32)
    b2 = np.asarray(b2, dtype=np.float32)

    B, T, D = x.shape
    N = B * T
    xf = x.reshape(N, D)

    # ---- router (host; 0.05% of model FLOPs — the sharding decision) ----
    logits = xf @ gate_w.T                           # [N, E]
    order = np.argsort(-logits, axis=1, kind="stable")
    i1, i2 = order[:, 0], order[:, 1]
    l1 = logits[np.arange(N), i1].astype(np.float64)
    l2 = logits[np.arange(N), i2].astype(np.float64)
    g1 = (1.0 / (1.0 + np.exp(l2 - l1))).astype(np.float32)
    g2 = (1.0 - g1).astype(np.float32)

    # ---- dispatch: gather per-expert tokens sorted by gate desc ----
    in_maps = []
    idx_per_e = []
    gv_per_e = []
    for e in range(E):
        sel1 = np.nonzero(i1 == e)[0]
        sel2 = np.nonzero(i2 == e)[0]
        idx = np.concatenate([sel1, sel2])
        gv = np.concatenate([g1[sel1], g2[sel2]])
        o = np.argsort(-gv, kind="stable")
        idx, gv = idx[o], gv[o]
        idx_per_e.append(idx)
        gv_per_e.append(gv)
        dev = min(idx.shape[0], C)

        xg = np.zeros((C, D), np.float32)
        xg[:dev] = xf[idx[:dev]]
        # [c, q, p, r, t]: xgt[c, q, p, r*512+t] = Xg[c*512+t, (4q+r)*128+p]
        xgt = np.ascontiguousarray(
            xg.T.reshape(2, 4, P, NCH, 512).transpose(3, 0, 2, 1, 4).reshape(
                NCH, 2, P, 4 * 512)).astype(np.float16)
        w1t = np.ascontiguousarray(
            w1[e].reshape(DC, P, HC, P).transpose(2, 1, 0, 3).reshape(
                HC, P, DC * P)).astype(np.float16)
        w2t = np.ascontiguousarray(
            w2[e].reshape(HC, P, DC, P).transpose(2, 1, 0, 3).reshape(
                DC, P, HC * P)).astype(np.float16)
        # DoubleRow fp8 w2: [dc, p, k, i, m] = e4m3(64*w2[(2k+i)*128+p, dc*128+m])
        w28t = np.ascontiguousarray(
            (w2[e] * SC).reshape(HC2, 2, P, DC, P).transpose(3, 2, 0, 1, 4)
        ).astype(E4M3)
        b1t = np.ascontiguousarray(b1[e].reshape(HC, P).T)
        in_maps.append({"xgt": xgt, "w1t": w1t, "w2t": w2t, "w28t": w28t,
                        "b1t": b1t})

    nc = _get_nc()
    res = bass_utils.run_bass_kernel_spmd(
        nc, in_maps, core_ids=list(range(N_CORES)), trace=TRACE)
    global LAST_RESULTS
    LAST_RESULTS = res

    # ---- combine (host): gate scale + top-2 sum; overflow tokens beyond
    # device capacity get their exact fp32 FFN here (~1% of pairs) ----
    out = np.zeros((N, D), np.float32)
    for e in range(E):
        idx = idx_per_e[e]
        gv = gv_per_e[e]
        dev = min(idx.shape[0], C)
        y = res.results[e]["ygt"][:, :dev].astype(np.float32).T  # [dev, D]
        out[idx[:dev]] += gv[:dev, None] * y
        if idx.shape[0] > C:
            xs = xf[idx[C:]]                                     # [S, D]
            hs = xs @ w1[e] + b1[e].reshape(1, D_HID)
            hs = 0.5 * hs * (1.0 + _erf(hs / np.sqrt(2.0)))
            ys = (hs @ w2[e]).astype(np.float32)
            out[idx[C:]] += gv[C:, None] * ys

    if np.any(b2):
        gate_full = np.zeros((N, E), np.float32)
        gate_full[np.arange(N), i1] = g1
        gate_full[np.arange(N), i2] = g2
        out += gate_full @ b2.reshape(E, D)

    return out.reshape(B, T, D)


# revision 7
# speedup vs baseline: 1.2717x; 1.0013x over previous
"""MoE FFN (E=8 experts, top-2) — expert-parallel Bass/Tile kernel for 8 TRN2 cores.

Strategy:
  - Host computes the (tiny) router: logits = x @ gate_w.T, top-2 per token,
    renormalized weights.  Token n is dispatched to cores e1(n), e2(n)
    (expert-parallel: core e holds expert e's weights).
  - Device capacity C=1024 tokens per expert (= the perfectly balanced
    N*K/E share).  Overflow tokens beyond 1024 per expert (~1.3% of
    pairs, pure load imbalance) are computed exactly on the host during
    combine.
  - Per-expert tokens are sorted by gate weight DESCENDING.  The first
    640 (large gates) take the fp16 path; the last 384 (gate <= ~0.46)
    use fp8e4m3 DoubleRow matmuls for mm2 (2x PE rate, measured 216ns
    for K=256/M=128/N=512 vs fp16's 216ns at half the MACs).  Their
    error (~3.4% RMS on y) is attenuated by the small gate: simulated
    end-to-end rel err 1.5e-2 vs the 2e-2 budget.
  - mm1 (fp16 for ALL tokens): hT[hc, c] = gelu(w1.T @ xgT + b1); 64
    chains of 8 matmuls.  Chunk c1's gelu eviction splits: cols 0:128
    (tokens 512:640) -> fp16 ht; cols 128:512 (tokens 640:1024) -> fp8
    ht in DoubleRow pair layout [p, 2, 384].
  - mm2 per dc: fp16 chains for token chunks [0:512) (N=512) and
    [512:640) (N=128) interleaved so each w2 tile's two LDWEIGHTS
    (2x97ns) hide under 213+53ns of matmul; then one fp8 DR chain of 16
    matmuls (K=256 each) over [640:1024) (N=384).  w2 ships both as
    fp16 tiles and as x64-scaled fp8 DR tiles (+4MB DMA, ~free at the
    measured ~430GB/s aggregate DMA bandwidth); the 1/64 is folded into
    the PSUM eviction scale.
  - 24 warmup matmuls on a memset tile start the PE at ~4us (vs ~10.5us
    first-DMA-ready), ramping the p-state clock while the head DMAs
    stream.
  - Engine assignment: PE = matmuls, scalar(Act) = gelu + w2/b1 DMA
    ring, sync(SP) = xg/w1/output DMA ring, vector(DVE) = PSUM
    evictions + warmup memset.
"""

import re

import numpy as np
import ml_dtypes

import bass_rust
import concourse.bass as bass
import concourse.mybir as mybir
import concourse.tile as tile
from concourse import bacc, bass_utils

P = 128
D_MODEL = 1024
D_HID = 4096
E = 8
TOP_K = 2
N_CORES = 8

DC = D_MODEL // P          # 8 d-chunks
HC = D_HID // P            # 32 h-chunks
HC2 = HC // 2              # 16 DoubleRow K-chunks (256 rows each)
C = 1024                   # device per-expert token capacity (rest -> host)
NCH = C // 512             # 2 token chunks of 512 (one PSUM bank each)
SPLIT = 640                # tokens [0:SPLIT) fp16 path, [SPLIT:C) fp8-mm2
NF8 = C - SPLIT            # 384 fp8 tokens
SC = 64.0                  # w2 fp8 pre-scale (lifts values out of denormals)
WARMUP = 30                # PE warmup matmuls (cover DMA head + pstate ramp)

F32 = mybir.dt.float32
F16 = mybir.dt.float16
F8 = mybir.dt.float8e4
MM_DT = F16
DR = mybir.MatmulPerfMode.DoubleRow
GELU = mybir.ActivationFunctionType.Gelu

W2_BUFS = 3                # w2 stream depth (tiles of [128, 4096])

E4M3 = ml_dtypes.float8_e4m3

_tail_patched = False


def _patch_light_tail():
    """Replace Tile's end-of-context machinery (multi-wait drain + two
    all-engine EVSEM barriers + semaphore range-clears, ~10us on HW) with
    single-wait drains on the sync engine covering every logical proc's final
    tick.  The NEFF is executed once per load in this flow, so semaphores
    need not be recycled."""
    global _tail_patched
    if _tail_patched:
        return
    _tail_patched = True

    def _drain_and_barrier(self, tick_clock, wait_clock):
        gc = tick_clock.global_clock
        ticks = eval(re.match(r"VectorClock\((.*)\)", repr(gc)).group(1))
        n = len(ticks)
        for i, v in enumerate(ticks):
            if v > 0:
                vc = bass_rust.VectorClock(
                    [v if j == i else 0 for j in range(n)])
                w = self.nc.sync.drain()
                wait_clock.add_sem_waits(
                    w.ins,
                    bass_rust.ScopedClock({None: vc}),
                    bass_rust.ScopedClock({}),
                )
        popped = self.nc._tile_sem_poison_stack.pop()
        assert popped is self._sem_poison
    tile.TileContext._drain_and_barrier = _drain_and_barrier


def build_nc():
    _patch_light_tail()
    nc = bacc.Bacc("TRN2", target_bir_lowering=False, debug=False,
                   num_devices=N_CORES)

    # Inputs, pre-tiled on host into consumption order (contiguous DMAs):
    #   xgt [NCH, 2, P, 2048]    xgt[c, q, p, r*512+t] = Xg[c*512+t, (4q+r)*128+p]
    #   w1t [HC, P, DC*P]        w1t[hc, p, dc*128+j] = w1[dc*128+p, hc*128+j]
    #   w2t [DC, P, HC*P]        w2t[dc, p, hc*128+j] = w2[hc*128+p, dc*128+j]
    #   w28t [DC, P, HC2, 2, P]  w28t[dc,p,k,i,j] = e4m3(64*w2[(2k+i)*128+p, dc*128+j])
    #   b1t [P, HC]              b1t[p, hc] = b1[hc*128+p]
    # Output:
    #   ygt [D, C]               ygt[d, n] = Y[n, d]   (pre-gate, fp16;
    #                            cols [SPLIT:) carry the exact value — the
    #                            x64 w2 scale is folded out at eviction)
    xgt = nc.dram_tensor("xgt", [NCH, 2, P, 4 * 512], MM_DT, kind="ExternalInput")
    w1t = nc.dram_tensor("w1t", [HC, P, DC * P], MM_DT, kind="ExternalInput")
    w2t = nc.dram_tensor("w2t", [DC, P, HC * P], MM_DT, kind="ExternalInput")
    w28t = nc.dram_tensor("w28t", [DC, P, HC2, 2, P], F8, kind="ExternalInput")
    b1t = nc.dram_tensor("b1t", [P, HC], F32, kind="ExternalInput")
    ygt = nc.dram_tensor("ygt", [D_MODEL, C], MM_DT, kind="ExternalOutput")

    with tile.TileContext(nc) as tc:
        with (
            tc.tile_pool(name="const", bufs=1) as const,
            tc.tile_pool(name="xg", bufs=1) as xg_pool,
            tc.tile_pool(name="w1", bufs=1) as w1_pool,
            tc.tile_pool(name="w2", bufs=W2_BUFS) as w2_pool,
            tc.tile_pool(name="w28", bufs=W2_BUFS) as w28_pool,
            tc.tile_pool(name="ht", bufs=1) as ht_pool,
            tc.tile_pool(name="yo", bufs=2) as yo_pool,
            # PSUM budget (8 banks of [128, 2KB]):
            #   ps1 3 (warmup + mm1 chains) + psf 1 (fp8 chain)
            #   + psa 2 (N=512 chains) + psb 1 (N=128 chains) = 7
            tc.tile_pool(name="ps1", bufs=3, space="PSUM") as ps1,
            tc.tile_pool(name="psf", bufs=1, space="PSUM") as psf,
            tc.tile_pool(name="psa", bufs=2, space="PSUM") as psa,
            tc.tile_pool(name="psb", bufs=1, space="PSUM") as psb,
        ):
            b1_sb = const.tile([P, HC], F32, name="b1sb")
            warm = const.tile([P, 512], F16, name="warm")

            # PE warmup: matmuls gated only on a DVE memset start the PE
            # early (vs ~10.5us DMA-head limited) and finish the p-state
            # ramp before real work.  The warm PSUM tile is never read.
            nc.vector.memset(warm[:], 0.0)
            for _ in range(WARMUP):
                # rotate through the ps1 pool (same name as the mm1
                # chains): 3 banks, no extra PSUM footprint, no
                # single-buffer WAW sem stalls
                psw = ps1.tile([P, 512], F32, name="ps1")
                nc.tensor.matmul(psw[:], lhsT=warm[:, :P], rhs=warm[:],
                                 start=True, stop=True)

            # Head DMAs: what pass A (token chunk c0) needs — w1#0 in two
            # halves plus the two 512KB xg-c0 quartets — split across the
            # two HWDGE rings in consumption order.
            xg_sb = {}
            w1_sb = {}

            def w1_load(hc, eng):
                t = w1_pool.tile([P, DC * P], MM_DT, name=f"w1_{hc}")
                eng.dma_start(out=t[:], in_=w1t[hc, :, :])
                w1_sb[hc] = t

            def xg_load(c, q, eng):
                t = xg_pool.tile([P, 4 * 512], MM_DT, name=f"xg{c}_{q}")
                eng.dma_start(out=t[:], in_=xgt[c, q, :, :])
                xg_sb[(c, q)] = t

            # First pieces cut small (subtile deps) so the first real
            # matmul waits on just 32KB of w1 + 128KB of xg.
            w1_sb[0] = w1_pool.tile([P, DC * P], MM_DT, name="w1_0")
            t = xg_pool.tile([P, 4 * 512], MM_DT, name="xg0_0")
            nc.scalar.dma_start(out=w1_sb[0][:, :P], in_=w1t[0, :, :P])
            nc.sync.dma_start(out=t[:, :512], in_=xgt[0, 0, :, :512])
            nc.scalar.dma_start(out=w1_sb[0][:, P:], in_=w1t[0, :, P:])
            nc.sync.dma_start(out=t[:, 512:], in_=xgt[0, 0, :, 512:])
            xg_sb[(0, 0)] = t
            nc.scalar.dma_start(out=b1_sb[:], in_=b1t[:, :])
            xg_load(0, 1, nc.sync)
            w1_load(2, nc.scalar)
            w1_load(1, nc.sync)
            w1_load(4, nc.scalar)
            w1_load(3, nc.sync)

            # ---- mm1: hT[hc, c] = gelu(w1.T @ xgT + b1), fp16 for all ----
            ht_c0 = {}                 # [128, 512] fp16  (tokens 0:512)
            ht_c1b = {}                # [128, 128] fp16  (tokens 512:640)
            ht8 = {}                   # [128, 2, 384] fp8 (tokens 640:1024)
            w2_sb = {}
            w28_sb = {}

            def w28_load(k, eng):
                t = w28_pool.tile([P, HC2, 2, P], F8, name="w28sb")
                eng.dma_start(out=t[:], in_=w28t[k, :, :, :, :])
                w28_sb[k] = t

            for c in range(NCH):
                for hc in range(HC):
                    if c == 0:
                        if hc + 5 < HC:
                            nhc = hc + 5
                            w1_load(nhc, nc.sync if nhc % 2 == 1
                                    else nc.scalar)
                        if hc == 4:
                            xg_load(1, 0, nc.sync)
                        if hc == 6:
                            xg_load(1, 1, nc.scalar)
                        if hc in (14, 19, 24):
                            k = {14: 0, 19: 1, 24: 2}[hc]
                            t = w2_pool.tile([P, HC * P], MM_DT,
                                             name="w2sb")
                            nc.scalar.dma_start(out=t[:], in_=w2t[k, :, :])
                            w2_sb[k] = t
                    else:
                        if hc in (8, 16, 24):
                            k = {8: 0, 16: 1, 24: 2}[hc]
                            w28_load(k, nc.scalar if hc != 16 else nc.sync)
                    ps = ps1.tile([P, 512], F32, name="ps1")
                    for dc in range(DC):
                        nc.tensor.matmul(
                            ps[:],
                            lhsT=w1_sb[hc][:, dc * P:(dc + 1) * P],
                            rhs=xg_sb[(c, dc // 4)][
                                :, (dc % 4) * 512:(dc % 4 + 1) * 512],
                            start=(dc == 0),
                            stop=(dc == DC - 1),
                        )
                    bias = b1_sb[:, hc:hc + 1]
                    if c == 0:
                        ht = ht_pool.tile([P, 512], MM_DT, name=f"ht{hc}")
                        nc.scalar.activation(ht[:], ps[:], GELU, bias=bias)
                        ht_c0[hc] = ht
                    else:
                        if hc % 2 == 0:
                            ht8[hc // 2] = ht_pool.tile(
                                [P, 2, NF8], F8, name=f"ht8_{hc // 2}")
                        htb = ht_pool.tile([P, P], MM_DT, name=f"htb{hc}")
                        nc.scalar.activation(htb[:], ps[:, :P], GELU,
                                             bias=bias)
                        ht_c1b[hc] = htb
                        nc.scalar.activation(ht8[hc // 2][:, hc % 2, :],
                                             ps[:, P:512], GELU, bias=bias)

            # ---- mm2: Y^T[dc] = sum_hc w2tile.T @ hT[hc] ----
            # fp16 N=512 + N=128 chains interleaved (shared lhsT), then
            # the fp8 DoubleRow N=384 chain (16 K=256 matmuls).
            for dc in range(DC):
                if dc + W2_BUFS < DC:
                    ndc = dc + W2_BUFS
                    t = w2_pool.tile([P, HC * P], MM_DT, name="w2sb")
                    nc.scalar.dma_start(out=t[:], in_=w2t[ndc, :, :])
                    w2_sb[ndc] = t
                    w28_load(ndc, nc.sync)
                def fp16_pair(dc):
                    ps_a = psa.tile([P, 512], F32, name="ps2a")
                    ps_b = psb.tile([P, 512], F32, name="ps2b")
                    for hc in range(HC):
                        l = w2_sb[dc][:, hc * P:(hc + 1) * P]
                        nc.tensor.matmul(ps_a[:], lhsT=l, rhs=ht_c0[hc][:],
                                         start=(hc == 0),
                                         stop=(hc == HC - 1))
                        nc.tensor.matmul(ps_b[:, :P], lhsT=l,
                                         rhs=ht_c1b[hc][:],
                                         start=(hc == 0),
                                         stop=(hc == HC - 1))
                    yo_a = yo_pool.tile([P, 512], MM_DT, name="yoa")
                    nc.vector.tensor_scalar_mul(yo_a[:], ps_a[:], 1.0)
                    nc.sync.dma_start(
                        out=ygt[dc * P:(dc + 1) * P, 0:512], in_=yo_a[:])
                    yo_b = yo_pool.tile([P, 512], MM_DT, name="yob")
                    nc.vector.tensor_scalar_mul(yo_b[:, :P], ps_b[:, :P],
                                                1.0)
                    nc.scalar.dma_start(
                        out=ygt[dc * P:(dc + 1) * P, 512:SPLIT],
                        in_=yo_b[:, :P])

                def fp8_chain(dc):
                    ps_f = psf.tile([P, 512], F32, name="ps2f")
                    for k in range(HC2):
                        nc.tensor.matmul(
                            ps_f[:, :NF8],
                            lhsT=w28_sb[dc][:, k, :, :],
                            rhs=ht8[k][:, :, :],
                            start=(k == 0), stop=(k == HC2 - 1),
                            perf_mode=DR)
                    yo_f = yo_pool.tile([P, 512], MM_DT, name="yof")
                    nc.vector.tensor_scalar_mul(yo_f[:, :NF8],
                                                ps_f[:, :NF8], 1.0 / SC)
                    nc.sync.dma_start(
                        out=ygt[dc * P:(dc + 1) * P, SPLIT:C],
                        in_=yo_f[:, :NF8])

                # last dc ends on the small fp16 eviction (short tail)
                if dc == DC - 1:
                    fp8_chain(dc)
                    fp16_pair(dc)
                else:
                    fp16_pair(dc)
                    fp8_chain(dc)
    nc.compile()
    return nc


_NC_CACHE = None
TRACE = False
LAST_RESULTS = None


def _get_nc():
    global _NC_CACHE
    if _NC_CACHE is None:
        _NC_CACHE = build_nc()
    return _NC_CACHE


def _erf(x):
    try:
        from scipy.special import erf
        return erf(x)
    except ImportError:
        import math
        return np.frompyfunc(math.erf, 1, 1)(x).astype(np.float64)


def kernel(x, gate_w, w1, b1, w2, b2):
    x = np.asarray(x, dtype=np.float32)
    gate_w = np.asarray(gate_w, dtype=np.float32)
    w1 = np.asarray(w1, dtype=np.float32)
    b1 = np.asarray(b1, dtype=np.float32)
    w2 = np.asarray(w2, dtype=np.float32)
    b2 = np.asarray(b2, dtype=np.float32)

    B, T, D = x.shape
    N = B * T
    xf = x.reshape(N, D)

    # ---- router (host; 0.05% of model FLOPs — the sharding decision) ----
    logits = xf @ gate_w.T                           # [N, E]
    order = np.argsort(-logits, axis=1, kind="stable")
    i1, i2 = order[:, 0], order[:, 1]
    l1 = logits[np.arange(N), i1].astype(np.float64)
    l2 = logits[np.arange(N), i2].astype(np.float64)
    g1 = (1.0 / (1.0 + np.exp(l2 - l1))).astype(np.float32)
    g2 = (1.0 - g1).astype(np.float32)

    # ---- dispatch: gather per-expert tokens sorted by gate desc ----
    in_maps = []
    idx_per_e = []
    gv_per_e = []
    for e in range(E):
        sel1 = np.nonzero(i1 == e)[0]
        sel2 = np.nonzero(i2 == e)[0]
        idx = np.concatenate([sel1, sel2])
        gv = np.concatenate([g1[sel1], g2[sel2]])
        o = np.argsort(-gv, kind="stable")
        idx, gv = idx[o], gv[o]
        idx_per_e.append(idx)
        gv_per_e.append(gv)
        dev = min(idx.shape[0], C)

        xg = np.zeros((C, D), np.float32)
        xg[:dev] = xf[idx[:dev]]
        # [c, q, p, r, t]: xgt[c, q, p, r*512+t] = Xg[c*512+t, (4q+r)*128+p]
        xgt = np.ascontiguousarray(
            xg.T.reshape(2, 4, P, NCH, 512).transpose(3, 0, 2, 1, 4).reshape(
                NCH, 2, P, 4 * 512)).astype(np.float16)
        w1t = np.ascontiguousarray(
            w1[e].reshape(DC, P, HC, P).transpose(2, 1, 0, 3).reshape(
                HC, P, DC * P)).astype(np.float16)
        w2t = np.ascontiguousarray(
            w2[e].reshape(HC, P, DC, P).transpose(2, 1, 0, 3).reshape(
                DC, P, HC * P)).astype(np.float16)
        # DoubleRow fp8 w2: [dc, p, k, i, m] = e4m3(64*w2[(2k+i)*128+p, dc*128+m])
        w28t = np.ascontiguousarray(
            (w2[e] * SC).reshape(HC2, 2, P, DC, P).transpose(3, 2, 0, 1, 4)
        ).astype(E4M3)
        b1t = np.ascontiguousarray(b1[e].reshape(HC, P).T)
        in_maps.append({"xgt": xgt, "w1t": w1t, "w2t": w2t, "w28t": w28t,
                        "b1t": b1t})

    nc = _get_nc()
    res = bass_utils.run_bass_kernel_spmd(
        nc, in_maps, core_ids=list(range(N_CORES)), trace=TRACE)
    global LAST_RESULTS
    LAST_RESULTS = res

    # ---- combine (host): gate scale + top-2 sum; overflow tokens beyond
    # device capacity get their exact fp32 FFN here (~1% of pairs) ----
    out = np.zeros((N, D), np.float32)
    for e in range(E):
        idx = idx_per_e[e]
        gv = gv_per_e[e]
        dev = min(idx.shape[0], C)
        y = res.results[e]["ygt"][:, :dev].astype(np.float32).T  # [dev, D]
        out[idx[:dev]] += gv[:dev, None] * y
        if idx.shape[0] > C:
            xs = xf[idx[C:]]                                     # [S, D]
            hs = xs @ w1[e] + b1[e].reshape(1, D_HID)
            hs = 0.5 * hs * (1.0 + _erf(hs / np.sqrt(2.0)))
            ys = (hs @ w2[e]).astype(np.float32)
            out[idx[C:]] += gv[C:, None] * ys

    if np.any(b2):
        gate_full = np.zeros((N, E), np.float32)
        gate_full[np.arange(N), i1] = g1
        gate_full[np.arange(N), i2] = g2
        out += gate_full @ b2.reshape(E, D)

    return out.reshape(B, T, D)


# revision 8
# speedup vs baseline: 1.2821x; 1.0082x over previous
"""MoE FFN (E=8 experts, top-2) — expert-parallel Bass/Tile kernel for 8 TRN2 cores.

Strategy:
  - Host computes the (tiny) router: logits = x @ gate_w.T, top-2 per token,
    renormalized weights.  Token n is dispatched to cores e1(n), e2(n)
    (expert-parallel: core e holds expert e's weights).
  - Device capacity C=1024 tokens per expert (= the perfectly balanced
    N*K/E share).  Overflow tokens beyond 1024 per expert (~1.3% of
    pairs, pure load imbalance) are computed exactly on the host during
    combine.
  - Per-expert tokens are sorted by gate weight DESCENDING.  The first
    640 (large gates) take the fp16 path; the last 384 (gate <= ~0.46)
    use fp8e4m3 DoubleRow matmuls for mm2 (2x PE rate, measured 216ns
    for K=256/M=128/N=512 vs fp16's 216ns at half the MACs).  Their
    error (~3.4% RMS on y) is attenuated by the small gate: simulated
    end-to-end rel err 1.5e-2 vs the 2e-2 budget.
  - mm1 (fp16 for ALL tokens): hT[hc, c] = gelu(w1.T @ xgT + b1); 64
    chains of 8 matmuls.  Chunk c1's gelu eviction splits: cols 0:128
    (tokens 512:640) -> fp16 ht; cols 128:512 (tokens 640:1024) -> fp8
    ht in DoubleRow pair layout [p, 2, 384].
  - mm2 per dc: fp16 chains for token chunks [0:512) (N=512) and
    [512:640) (N=128) interleaved so each w2 tile's two LDWEIGHTS
    (2x97ns) hide under 213+53ns of matmul; then one fp8 DR chain of 16
    matmuls (K=256 each) over [640:1024) (N=384).  w2 ships both as
    fp16 tiles and as x64-scaled fp8 DR tiles (+4MB DMA, ~free at the
    measured ~430GB/s aggregate DMA bandwidth); the 1/64 is folded into
    the PSUM eviction scale.
  - 24 warmup matmuls on a memset tile start the PE at ~4us (vs ~10.5us
    first-DMA-ready), ramping the p-state clock while the head DMAs
    stream.
  - Engine assignment: PE = matmuls, scalar(Act) = gelu + w2/b1 DMA
    ring, sync(SP) = xg/w1/output DMA ring, vector(DVE) = PSUM
    evictions + warmup memset.
"""

import re

import numpy as np
import ml_dtypes

import bass_rust
import concourse.bass as bass
import concourse.mybir as mybir
import concourse.tile as tile
from concourse import bacc, bass_utils

P = 128
D_MODEL = 1024
D_HID = 4096
E = 8
TOP_K = 2
N_CORES = 8

DC = D_MODEL // P          # 8 d-chunks
HC = D_HID // P            # 32 h-chunks
HC2 = HC // 2              # 16 DoubleRow K-chunks (256 rows each)
C = 1024                   # device per-expert token capacity (rest -> host)
NCH = C // 512             # 2 token chunks of 512 (one PSUM bank each)
SPLIT = 640                # tokens [0:SPLIT) fp16 path, [SPLIT:C) fp8-mm2
NF8 = C - SPLIT            # 384 fp8 tokens
SC = 64.0                  # w2 fp8 pre-scale (lifts values out of denormals)
WARMUP = 14                # PE warmup matmuls (cover DMA head + pstate ramp)

F32 = mybir.dt.float32
F16 = mybir.dt.float16
F8 = mybir.dt.float8e4
MM_DT = F16
DR = mybir.MatmulPerfMode.DoubleRow
GELU = mybir.ActivationFunctionType.Gelu

W2_BUFS = 3                # w2 stream depth (tiles of [128, 4096])

E4M3 = ml_dtypes.float8_e4m3

_tail_patched = False


def _patch_light_tail():
    """Replace Tile's end-of-context machinery (multi-wait drain + two
    all-engine EVSEM barriers + semaphore range-clears, ~10us on HW) with
    single-wait drains on the sync engine covering every logical proc's final
    tick.  The NEFF is executed once per load in this flow, so semaphores
    need not be recycled."""
    global _tail_patched
    if _tail_patched:
        return
    _tail_patched = True

    def _drain_and_barrier(self, tick_clock, wait_clock):
        gc = tick_clock.global_clock
        ticks = eval(re.match(r"VectorClock\((.*)\)", repr(gc)).group(1))
        n = len(ticks)
        for i, v in enumerate(ticks):
            if v > 0:
                vc = bass_rust.VectorClock(
                    [v if j == i else 0 for j in range(n)])
                w = self.nc.sync.drain()
                wait_clock.add_sem_waits(
                    w.ins,
                    bass_rust.ScopedClock({None: vc}),
                    bass_rust.ScopedClock({}),
                )
        popped = self.nc._tile_sem_poison_stack.pop()
        assert popped is self._sem_poison
    tile.TileContext._drain_and_barrier = _drain_and_barrier


def build_nc():
    _patch_light_tail()
    nc = bacc.Bacc("TRN2", target_bir_lowering=False, debug=False,
                   num_devices=N_CORES)

    # Inputs, pre-tiled on host into consumption order (contiguous DMAs):
    #   xgt [NCH, 2, P, 2048]    xgt[c, q, p, r*512+t] = Xg[c*512+t, (4q+r)*128+p]
    #   w1t [HC, P, DC*P]        w1t[hc, p, dc*128+j] = w1[dc*128+p, hc*128+j]
    #   w2t [DC, P, HC*P]        w2t[dc, p, hc*128+j] = w2[hc*128+p, dc*128+j]
    #   w28t [DC, P, HC2, 2, P]  w28t[dc,p,k,i,j] = e4m3(64*w2[(2k+i)*128+p, dc*128+j])
    #   b1t [P, HC]              b1t[p, hc] = b1[hc*128+p]
    # Output:
    #   ygt [D, C]               ygt[d, n] = Y[n, d]   (pre-gate, fp16;
    #                            cols [SPLIT:) carry the exact value — the
    #                            x64 w2 scale is folded out at eviction)
    xgt = nc.dram_tensor("xgt", [NCH, 2, P, 4 * 512], MM_DT, kind="ExternalInput")
    w1t = nc.dram_tensor("w1t", [HC, P, DC * P], MM_DT, kind="ExternalInput")
    w2t = nc.dram_tensor("w2t", [DC, P, HC * P], MM_DT, kind="ExternalInput")
    w28t = nc.dram_tensor("w28t", [DC, P, HC2, 2, P], F8, kind="ExternalInput")
    b1t = nc.dram_tensor("b1t", [P, HC], F32, kind="ExternalInput")
    ygt = nc.dram_tensor("ygt", [D_MODEL, C], MM_DT, kind="ExternalOutput")

    with tile.TileContext(nc) as tc:
        with (
            tc.tile_pool(name="const", bufs=1) as const,
            tc.tile_pool(name="xg", bufs=1) as xg_pool,
            tc.tile_pool(name="w1", bufs=1) as w1_pool,
            tc.tile_pool(name="w2", bufs=W2_BUFS) as w2_pool,
            tc.tile_pool(name="w28", bufs=W2_BUFS) as w28_pool,
            tc.tile_pool(name="ht", bufs=1) as ht_pool,
            tc.tile_pool(name="yo", bufs=2) as yo_pool,
            # PSUM budget (8 banks of [128, 2KB]):
            #   ps1 3 (warmup + mm1 chains) + psf 1 (fp8 chain)
            #   + psa 2 (N=512 chains) + psb 1 (N=128 chains) = 7
            tc.tile_pool(name="ps1", bufs=3, space="PSUM") as ps1,
            tc.tile_pool(name="psf", bufs=1, space="PSUM") as psf,
            tc.tile_pool(name="psa", bufs=2, space="PSUM") as psa,
            tc.tile_pool(name="psb", bufs=1, space="PSUM") as psb,
        ):
            b1_sb = const.tile([P, HC], F32, name="b1sb")
            warm = const.tile([P, 512], F16, name="warm")

            # PE warmup: matmuls gated only on a DVE memset start the PE
            # early (vs ~10.5us DMA-head limited) and finish the p-state
            # ramp before real work.  The warm PSUM tile is never read.
            nc.vector.memset(warm[:], 0.0)
            for _ in range(WARMUP):
                # rotate through the ps1 pool (same name as the mm1
                # chains): 3 banks, no extra PSUM footprint, no
                # single-buffer WAW sem stalls
                psw = ps1.tile([P, 512], F32, name="ps1")
                nc.tensor.matmul(psw[:], lhsT=warm[:, :P], rhs=warm[:],
                                 start=True, stop=True)

            # Head DMAs: what pass A (token chunk c0) needs — w1#0 in two
            # halves plus the two 512KB xg-c0 quartets — split across the
            # two HWDGE rings in consumption order.
            xg_sb = {}
            w1_sb = {}

            def w1_load(hc, eng):
                t = w1_pool.tile([P, DC * P], MM_DT, name=f"w1_{hc}")
                eng.dma_start(out=t[:], in_=w1t[hc, :, :])
                w1_sb[hc] = t

            def xg_load(c, q, eng):
                t = xg_pool.tile([P, 4 * 512], MM_DT, name=f"xg{c}_{q}")
                eng.dma_start(out=t[:], in_=xgt[c, q, :, :])
                xg_sb[(c, q)] = t

            # First pieces cut small (subtile deps) so the first real
            # matmul waits on just 32KB of w1 + 128KB of xg.
            w1_sb[0] = w1_pool.tile([P, DC * P], MM_DT, name="w1_0")
            t = xg_pool.tile([P, 4 * 512], MM_DT, name="xg0_0")
            nc.scalar.dma_start(out=w1_sb[0][:, :P], in_=w1t[0, :, :P])
            nc.sync.dma_start(out=t[:, :512], in_=xgt[0, 0, :, :512])
            nc.scalar.dma_start(out=w1_sb[0][:, P:], in_=w1t[0, :, P:])
            nc.sync.dma_start(out=t[:, 512:], in_=xgt[0, 0, :, 512:])
            xg_sb[(0, 0)] = t
            # xg(0,1) on the scalar ring so the sync ring feeds the w1
            # stream without a 1MB head-of-line stall
            xg_load(0, 1, nc.scalar)
            w1_load(1, nc.sync)
            nc.scalar.dma_start(out=b1_sb[:], in_=b1t[:, :])
            w1_load(3, nc.sync)
            w1_load(2, nc.scalar)
            w1_load(4, nc.scalar)

            # ---- mm1: hT[hc, c] = gelu(w1.T @ xgT + b1), fp16 for all ----
            ht_c0 = {}                 # [128, 512] fp16  (tokens 0:512)
            ht_c1b = {}                # [128, 128] fp16  (tokens 512:640)
            ht8 = {}                   # [128, 2, 384] fp8 (tokens 640:1024)
            w2_sb = {}
            w28_sb = {}

            def w28_load(k, eng):
                t = w28_pool.tile([P, HC2, 2, P], F8, name="w28sb")
                eng.dma_start(out=t[:], in_=w28t[k, :, :, :, :])
                w28_sb[k] = t

            for c in range(NCH):
                for hc in range(HC):
                    if c == 0:
                        if hc + 5 < HC:
                            nhc = hc + 5
                            w1_load(nhc, nc.sync if nhc % 2 == 1
                                    else nc.scalar)
                        if hc == 4:
                            xg_load(1, 0, nc.sync)
                        if hc == 6:
                            xg_load(1, 1, nc.scalar)
                        if hc in (14, 19, 24):
                            k = {14: 0, 19: 1, 24: 2}[hc]
                            t = w2_pool.tile([P, HC * P], MM_DT,
                                             name="w2sb")
                            nc.scalar.dma_start(out=t[:], in_=w2t[k, :, :])
                            w2_sb[k] = t
                    else:
                        if hc in (8, 16, 24):
                            k = {8: 0, 16: 1, 24: 2}[hc]
                            w28_load(k, nc.scalar if hc != 16 else nc.sync)
                    ps = ps1.tile([P, 512], F32, name="ps1")
                    for dc in range(DC):
                        nc.tensor.matmul(
                            ps[:],
                            lhsT=w1_sb[hc][:, dc * P:(dc + 1) * P],
                            rhs=xg_sb[(c, dc // 4)][
                                :, (dc % 4) * 512:(dc % 4 + 1) * 512],
                            start=(dc == 0),
                            stop=(dc == DC - 1),
                        )
                    bias = b1_sb[:, hc:hc + 1]
                    if c == 0:
                        ht = ht_pool.tile([P, 512], MM_DT, name=f"ht{hc}")
                        nc.scalar.activation(ht[:], ps[:], GELU, bias=bias)
                        ht_c0[hc] = ht
                    else:
                        if hc % 2 == 0:
                            ht8[hc // 2] = ht_pool.tile(
                                [P, 2, NF8], F8, name=f"ht8_{hc // 2}")
                        htb = ht_pool.tile([P, P], MM_DT, name=f"htb{hc}")
                        nc.scalar.activation(htb[:], ps[:, :P], GELU,
                                             bias=bias)
                        ht_c1b[hc] = htb
                        nc.scalar.activation(ht8[hc // 2][:, hc % 2, :],
                                             ps[:, P:512], GELU, bias=bias)

            # ---- mm2: Y^T[dc] = sum_hc w2tile.T @ hT[hc] ----
            # fp16 N=512 + N=128 chains interleaved (shared lhsT), then
            # the fp8 DoubleRow N=384 chain (16 K=256 matmuls).
            for dc in range(DC):
                if dc + W2_BUFS < DC:
                    ndc = dc + W2_BUFS
                    t = w2_pool.tile([P, HC * P], MM_DT, name="w2sb")
                    nc.scalar.dma_start(out=t[:], in_=w2t[ndc, :, :])
                    w2_sb[ndc] = t
                    w28_load(ndc, nc.sync)
                def fp16_pair(dc):
                    ps_a = psa.tile([P, 512], F32, name="ps2a")
                    ps_b = psb.tile([P, 512], F32, name="ps2b")
                    for hc in range(HC):
                        l = w2_sb[dc][:, hc * P:(hc + 1) * P]
                        nc.tensor.matmul(ps_a[:], lhsT=l, rhs=ht_c0[hc][:],
                                         start=(hc == 0),
                                         stop=(hc == HC - 1))
                        nc.tensor.matmul(ps_b[:, :P], lhsT=l,
                                         rhs=ht_c1b[hc][:],
                                         start=(hc == 0),
                                         stop=(hc == HC - 1))
                    yo_a = yo_pool.tile([P, 512], MM_DT, name="yoa")
                    nc.vector.tensor_scalar_mul(yo_a[:], ps_a[:], 1.0)
                    nc.sync.dma_start(
                        out=ygt[dc * P:(dc + 1) * P, 0:512], in_=yo_a[:])
                    yo_b = yo_pool.tile([P, 512], MM_DT, name="yob")
                    nc.vector.tensor_scalar_mul(yo_b[:, :P], ps_b[:, :P],
                                                1.0)
                    nc.scalar.dma_start(
                        out=ygt[dc * P:(dc + 1) * P, 512:SPLIT],
                        in_=yo_b[:, :P])

                def fp8_chain(dc):
                    ps_f = psf.tile([P, 512], F32, name="ps2f")
                    for k in range(HC2):
                        nc.tensor.matmul(
                            ps_f[:, :NF8],
                            lhsT=w28_sb[dc][:, k, :, :],
                            rhs=ht8[k][:, :, :],
                            start=(k == 0), stop=(k == HC2 - 1),
                            perf_mode=DR)
                    yo_f = yo_pool.tile([P, 512], MM_DT, name="yof")
                    nc.vector.tensor_scalar_mul(yo_f[:, :NF8],
                                                ps_f[:, :NF8], 1.0 / SC)
                    nc.sync.dma_start(
                        out=ygt[dc * P:(dc + 1) * P, SPLIT:C],
                        in_=yo_f[:, :NF8])

                # last dc ends on the small fp16 eviction (short tail)
                if dc == DC - 1:
                    fp8_chain(dc)
                    fp16_pair(dc)
                else:
                    fp16_pair(dc)
                    fp8_chain(dc)
    nc.compile()
    return nc


_NC_CACHE = None
TRACE = False
LAST_RESULTS = None


def _get_nc():
    global _NC_CACHE
    if _NC_CACHE is None:
        _NC_CACHE = build_nc()
    return _NC_CACHE


def _erf(x):
    try:
        from scipy.special import erf
        return erf(x)
    except ImportError:
        import math
        return np.frompyfunc(math.erf, 1, 1)(x).astype(np.float64)


def kernel(x, gate_w, w1, b1, w2, b2):
    x = np.asarray(x, dtype=np.float32)
    gate_w = np.asarray(gate_w, dtype=np.float32)
    w1 = np.asarray(w1, dtype=np.float32)
    b1 = np.asarray(b1, dtype=np.float32)
    w2 = np.asarray(w2, dtype=np.float32)
    b2 = np.asarray(b2, dtype=np.float32)

    B, T, D = x.shape
    N = B * T
    xf = x.reshape(N, D)

    # ---- router (host; 0.05% of model FLOPs — the sharding decision) ----
    logits = xf @ gate_w.T                           # [N, E]
    order = np.argsort(-logits, axis=1, kind="stable")
    i1, i2 = order[:, 0], order[:, 1]
    l1 = logits[np.arange(N), i1].astype(np.float64)
    l2 = logits[np.arange(N), i2].astype(np.float64)
    g1 = (1.0 / (1.0 + np.exp(l2 - l1))).astype(np.float32)
    g2 = (1.0 - g1).astype(np.float32)

    # ---- dispatch: gather per-expert tokens sorted by gate desc ----
    in_maps = []
    idx_per_e = []
    gv_per_e = []
    for e in range(E):
        sel1 = np.nonzero(i1 == e)[0]
        sel2 = np.nonzero(i2 == e)[0]
        idx = np.concatenate([sel1, sel2])
        gv = np.concatenate([g1[sel1], g2[sel2]])
        o = np.argsort(-gv, kind="stable")
        idx, gv = idx[o], gv[o]
        idx_per_e.append(idx)
        gv_per_e.append(gv)
        dev = min(idx.shape[0], C)

        xg = np.zeros((C, D), np.float32)
        xg[:dev] = xf[idx[:dev]]
        # [c, q, p, r, t]: xgt[c, q, p, r*512+t] = Xg[c*512+t, (4q+r)*128+p]
        xgt = np.ascontiguousarray(
            xg.T.reshape(2, 4, P, NCH, 512).transpose(3, 0, 2, 1, 4).reshape(
                NCH, 2, P, 4 * 512)).astype(np.float16)
        w1t = np.ascontiguousarray(
            w1[e].reshape(DC, P, HC, P).transpose(2, 1, 0, 3).reshape(
                HC, P, DC * P)).astype(np.float16)
        w2t = np.ascontiguousarray(
            w2[e].reshape(HC, P, DC, P).transpose(2, 1, 0, 3).reshape(
                DC, P, HC * P)).astype(np.float16)
        # DoubleRow fp8 w2: [dc, p, k, i, m] = e4m3(64*w2[(2k+i)*128+p, dc*128+m])
        w28t = np.ascontiguousarray(
            (w2[e] * SC).reshape(HC2, 2, P, DC, P).transpose(3, 2, 0, 1, 4)
        ).astype(E4M3)
        b1t = np.ascontiguousarray(b1[e].reshape(HC, P).T)
        in_maps.append({"xgt": xgt, "w1t": w1t, "w2t": w2t, "w28t": w28t,
                        "b1t": b1t})

    nc = _get_nc()
    res = bass_utils.run_bass_kernel_spmd(
        nc, in_maps, core_ids=list(range(N_CORES)), trace=TRACE)
    global LAST_RESULTS
    LAST_RESULTS = res

    # ---- combine (host): gate scale + top-2 sum; overflow tokens beyond
    # device capacity get their exact fp32 FFN here (~1% of pairs) ----
    out = np.zeros((N, D), np.float32)
    for e in range(E):
        idx = idx_per_e[e]
        gv = gv_per_e[e]
        dev = min(idx.shape[0], C)
        y = res.results[e]["ygt"][:, :dev].astype(np.float32).T  # [dev, D]
        out[idx[:dev]] += gv[:dev, None] * y
        if idx.shape[0] > C:
            xs = xf[idx[C:]]                                     # [S, D]
            hs = xs @ w1[e] + b1[e].reshape(1, D_HID)
            hs = 0.5 * hs * (1.0 + _erf(hs / np.sqrt(2.0)))
            ys = (hs @ w2[e]).astype(np.float32)
            out[idx[C:]] += gv[C:, None] * ys

    if np.any(b2):
        gate_full = np.zeros((N, E), np.float32)
        gate_full[np.arange(N), i1] = g1
        gate_full[np.arange(N), i2] = g2
        out += gate_full @ b2.reshape(E, D)

    return out.reshape(B, T, D)
